# revision 15
# baseline (speedup 1.0000x reference)
"""Euclidean distance block (retrieval kNN) on 8 TRN2 NeuronCores.

dist[b, s, p] = sqrt(sum_c (x1[b, c, p] - x2[b, s, c, p])^2)   p = spatial (h*w)
out[b] = dist[b].reshape(S * h * w)

Sharding: data-parallel over batch B=32 -> 4 batches per core, no comms.

Design (v3; baseline f32/SWDGE was ~145-166us traced, v2 ~120us):

1. HOST-SIDE bf16 STAGING. The baseline streamed x2 as f32 (45 MB/core) and
   cast f32->bf16 on the SWDGE ring; the subtract was already bf16, so
   pre-casting x2/x1 to bf16 on the host gives identical numerics with HALF
   the HBM read traffic (22.6 MB/core) and removes the cast -> every load is
   a plain HWDGE DMA (sync ring, ~0.6us first byte, no ~6us Q7 warmup).
   x1 is also pre-duplicated on host to [128=(2x64c), BL, HW] so the kernel
   needs no SBUF->SBUF partition duplicate. Output is stored bf16 and
   upcast to f32 on host (rel err budget 2e-2, bf16 adds <0.4%).

2. DOUBLE-PAIR PIPELINE. SBUF partitions carry (support_pair, channel) =
   2*64 = 128. Each DMA covers TWO support pairs [128, 2, HW] (902KB), and
   one DVE subtract (bf16 2x mode, x1 broadcast over the pair dim via a
   stride-0 AP) plus one square (4 doubles on ACT, 2 on DVE per batch;
   GpSimd tensor ops measured 5x slow - never use) process both pairs:
   halving the instruction count halves the per-op dependency/sem latency
   that showed up as 15-20% engine idle at pair granularity. PE mask-
   matmuls accumulate per-support sums over C into [25, 441] PSUM tiles
   (4 spatial quarters), ACT sqrt -> bf16 store on the scalar HWDGE ring
   (loads and stores never share a FIFO). x1 lives in one tile per batch
   (a shared tile's slice writes would WAR-serialize against every
   in-flight subtract read).

3. PE KEEP-WARM FILLERS. TRN2's power manager runs the PE at HALF clock
   (371ns per 441-col matmul) unless it has been continuously busy for
   ~3.4us, full clock (188ns) after. Per-pair bursts (~1.5us) with gaps
   never promote. Fillers = matmuls of a zeroed SBUF tile with whatever
   weights are resident (ldweights=False) accumulated into live PSUM:
   adds 0.0, costs no weight reload, keeps the PE promoted.

4. LDWEIGHTS ELISION. The 4 quarter-matmuls of a pair share one mask; only
   quarter 0 self-loads weights (ldweights=False on the rest elides the
   ~101ns InstLdweights each). _verify_ldw_order() walks the final BIR and
   asserts no foreign weight load lands between a loader and its dependents
   (the Tile scheduler could in principle reorder same-engine matmuls).

5. SHORT TAIL. The last batch computes the half-width leftover support 24
   LAST, quarter-sliced: its 4x56KB loads are the final DMAs and each
   quarter's sub->square->matmul(stop)->sqrt->store chain fires as its
   56KB lands, so the post-last-byte critical path is one 441-wide chain.
"""

import numpy as np

B, S, C, H, W = 32, 25, 64, 42, 42
HW = H * W            # 1764
NCORES = 8
BL = B // NCORES      # 4 batches per core
NPAIR = 12            # full support pairs (24 supports); support 24 leftover
NQ = 4                # spatial quarters
QW = HW // NQ         # 441
NMASK = 14            # 12 pair masks + leftover mask (12) + zero filler (13)
LO = 12               # mask index of the leftover support
ZW = 13               # mask index of the all-zero filler weights

NFILL = 4             # keep-warm fillers per double-pair
FILLW = 441           # filler matmul moving columns
ELIDE_LDW = False     # legalization re-pairs an InstLdweights with every
                      # matmult regardless; LDW overlaps MM execution anyway

# square-engine schedule per double-pair i (A=ACT, D=DVE mult)
SQ_ENG = "ADAADA"
SQ_ENG_LAST = "AADAAD"
NDBL = NPAIR // 2     # double-pairs per batch

_cache = {}


def _build_nc():
    import concourse.bacc as bacc
    import concourse.mybir as mybir
    from concourse.tile import TileContext
    from concourse.bass import MemorySpace, broadcast_tensor_aps

    f32 = mybir.dt.float32
    bf16 = mybir.dt.bfloat16
    Square = mybir.ActivationFunctionType.Square
    Sqrt = mybir.ActivationFunctionType.Sqrt
    sub = mybir.AluOpType.subtract
    mul = mybir.AluOpType.mult

    # Square and Sqrt both live in the "sqrt_and_others" act-function set,
    # but the table-load chooser picks the first set containing each one,
    # alternating two ~2.7us table reloads per batch. Strip the two
    # functions from every other set (contents only - set ids are
    # positional) so one resident table serves the whole kernel.
    _orig_tables = bacc.get_activation_tables

    def _pinned_tables(arch):
        t = _orig_tables(arch)
        for name, fns in t.items():
            if name != "sqrt_and_others":
                fns.discard(Square)
                fns.discard(Sqrt)
        return t

    bacc.get_activation_tables = _pinned_tables
    nc = bacc.Bacc()
    x1 = nc.declare_dram_parameter("x1", [128, BL, HW], bf16, isOutput=False)
    x2 = nc.declare_dram_parameter("x2", [BL, NDBL, 128, 2 * HW], bf16, isOutput=False)
    x2lo = nc.declare_dram_parameter("x2lo", [BL, 64, HW], bf16, isOutput=False)
    mk = nc.declare_dram_parameter("mask", [NMASK, 128, S], bf16, isOutput=False)
    out = nc.declare_dram_parameter("out", [BL, S * HW], bf16, isOutput=True)

    # build-time bookkeeping for _verify_ldw_order
    elide_owner = {}      # elided matmult name -> its weight-loader's name
    filler_names = set()

    def mm(pst_q, w, mov, start, stop, loader=None, skip=False):
        inst = nc.tensor.matmul(
            pst_q, w, mov, start=start, stop=stop, skip_group_check=skip
        )
        if loader is not None and ELIDE_LDW:
            inst.ins.ldweights = False
            elide_owner[inst.ins.name] = loader.ins.name
        return inst

    with TileContext(nc) as tc:
        with (
            tc.tile_pool(name="x2p", bufs=6) as x2p,
            tc.tile_pool(name="lop", bufs=2) as lop,
            tc.tile_pool(name="sqp", bufs=4) as sqp,
            tc.tile_pool(name="sqlp", bufs=2) as sqlp,
            tc.tile_pool(name="x1p", bufs=4) as x1p,
            tc.tile_pool(name="outp", bufs=2) as outp,
            tc.tile_pool(name="cst", bufs=1) as cst,
            tc.tile_pool(name="ps", bufs=2, space=MemorySpace.PSUM) as psp,
        ):
            mt = cst.tile([128, NMASK, S], bf16)
            nc.scalar.dma_start(mt[:], mk.rearrange("g k m -> k g m"))

            zt = cst.tile([128, FILLW], bf16, name="zt")
            nc.vector.memset(zt[:], 0.0)

            x1bt = [x1p.tile([128, HW], bf16, name=f"x1b{b}", tag="x1") for b in range(BL)]
            nc.scalar.dma_start(x1bt[0][:], x1[:, 0, :])

            def filler(pst, j, n=NFILL):
                # zero-data accumulates with whatever weights are resident:
                # keeps the PE busy through the per-pair DMA gap so the power
                # manager holds full clock; adds 0.0 to live PSUM
                for k in range(n):
                    inst = nc.tensor.matmul(
                        pst[(j + k) % NQ][:, :],
                        mt[:, ZW, :],
                        zt[:, :],
                        start=False,
                        stop=False,
                        skip_group_check=True,
                    )
                    inst.ins.ldweights = False
                    filler_names.add(inst.ins.name)

            for b in range(BL):
                last = b == BL - 1
                # last batch: front-load ACT squares so ACT's queue is empty
                # when the end-chain (lo squares + sqrts) arrives
                sq_eng = SQ_ENG_LAST if last else SQ_ENG

                pst = [
                    psp.tile([S, QW], f32, name=f"ps{q}", tag=f"ps{q}")
                    for q in range(NQ)
                ]

                # work groups: (first_pair_j, n_pairs, sq_engine). Batch 0
                # opens with two single pairs so the first subtract fires
                # ~2.5us earlier (half the first DMA's wire time).
                if b == 0:
                    groups = [(0, 1, "A"), (1, 1, "D")] + [
                        (2 * i, 2, sq_eng[i]) for i in range(1, NDBL)
                    ]
                else:
                    groups = [(2 * i, 2, sq_eng[i]) for i in range(NDBL)]

                # loads: groups, then leftover; batch b+1's x1 rides last
                # (DVE backlog is largest at batch end, hiding the cadence gap)
                dbls = []
                for gi, (j0, np_, _) in enumerate(groups):
                    x2t = x2p.tile([128, np_, HW], bf16, tag="x2t")
                    src = x2[b, j0 // 2].rearrange("k (pp p) -> k pp p", pp=2)
                    pp0 = j0 % 2
                    ring = nc.sync if gi % 2 == 0 else nc.scalar
                    ring.dma_start(x2t[:], src[:, pp0 : pp0 + np_, :])
                    dbls.append(x2t)
                x2l = lop.tile([64, HW], bf16, tag="lo")
                if not last:
                    nc.sync.dma_start(x2l[:], x2lo[b])
                    nc.scalar.dma_start(x1bt[b + 1][:], x1[:, b + 1, :])
                else:
                    # leftover is the kernel tail: quarter-sliced, loaded last
                    for q in range(NQ):
                        nc.sync.dma_start(
                            x2l[:, q * QW : (q + 1) * QW],
                            x2lo[b][:, q * QW : (q + 1) * QW],
                        )

                for gi, (j0, np_, eng) in enumerate(groups):
                    x2t = dbls[gi]
                    if np_ == 2:
                        x1u = x1bt[b][:].rearrange("k (u p) -> k u p", u=1)
                        i0, i1 = broadcast_tensor_aps(x2t[:, :, :], x1u)
                    else:
                        i0, i1 = x2t[:, 0, :], x1bt[b][:]
                    nc.vector.tensor_tensor(i0, i0, i1, sub)
                    sq = sqp.tile([128, 2, HW], bf16, tag="sq")
                    if eng == "A":
                        nc.scalar.activation(sq[:, :np_, :], x2t[:], Square)
                    else:
                        nc.vector.tensor_tensor(sq[:, :np_, :], x2t[:], x2t[:], mul)
                    for pi in range(np_):
                        j = j0 + pi
                        for q in range(NQ):
                            mm(
                                pst[q][:, :],
                                mt[:, j, :],
                                sq[:, pi, q * QW : (q + 1) * QW],
                                start=(j == 0),
                                stop=False,
                            )
                    if not last:
                        filler(pst, gi)

                # leftover support 24 last: a short half-width end-chain per
                # batch; its sqrt/store overlaps the next batch's stream
                ot = outp.tile([S, HW], bf16, name="ot", tag="ot")
                sql = sqlp.tile([64, HW], bf16, name="sql", tag="sql")
                if not last:
                    nc.vector.tensor_tensor(x2l[:], x2l[:], x1bt[b][0:64, :], sub)
                    nc.scalar.activation(sql[:], x2l[:], Square)
                    lo_loader = None
                    for q in range(NQ):
                        inst = mm(
                            pst[q][:, :],
                            mt[0:64, LO, :],
                            sql[:, q * QW : (q + 1) * QW],
                            start=False,
                            stop=True,
                            loader=lo_loader,
                        )
                        if lo_loader is None:
                            lo_loader = inst
                    filler(pst, 0, 2)
                    for q in range(NQ):
                        nc.scalar.activation(
                            ot[:, q * QW : (q + 1) * QW], pst[q][:], Sqrt
                        )
                    nc.scalar.dma_start(out[b].rearrange("(s p) -> s p", s=S), ot[:])
                else:
                    # tail: leftover quarters stream in as the final DMAs;
                    # each quarter's chain fires on its own 56KB completion
                    lo_loader = None
                    for q in range(NQ):
                        qs = slice(q * QW, (q + 1) * QW)
                        nc.vector.tensor_tensor(
                            x2l[:, qs], x2l[:, qs], x1bt[b][0:64, qs], sub
                        )
                        nc.scalar.activation(sql[:, qs], x2l[:, qs], Square)
                        inst = mm(
                            pst[q][:, :],
                            mt[0:64, LO, :],
                            sql[:, qs],
                            start=False,
                            stop=True,
                            loader=lo_loader,
                        )
                        if lo_loader is None:
                            lo_loader = inst
                        nc.scalar.activation(ot[:, qs], pst[q][:], Sqrt)
                        nc.scalar.dma_start(
                            out[b].rearrange("(s p) -> s p", s=S)[:, qs], ot[:, qs]
                        )

    try:
        nc.finalize()
    finally:
        bacc.get_activation_tables = _orig_tables
    if ELIDE_LDW:
        _verify_ldw_order(nc, elide_owner, filler_names)
    return nc


def _verify_ldw_order(nc, elide_owner, filler_names):
    """The 4 quarter-matmuls of a pair share one weight load. Walk the final
    (post-Tile-scheduling) program order and assert no other weight-loading
    matmult lands between a loader and its elided dependents."""
    last_loader = None
    for blk in nc.m.functions[0].blocks:
        for inst in blk.instructions:
            if type(inst).__name__ != "InstMatmult":
                continue
            name = inst.name
            if name in filler_names:
                continue  # zero moving data: any weights give 0
            if name in elide_owner:
                if last_loader != elide_owner[name]:
                    raise RuntimeError(
                        f"ldweights elision unsafe: {name} expects weights of "
                        f"{elide_owner[name]} but last loader is {last_loader}"
                    )
            else:
                last_loader = name


def get_nc():
    if "nc" not in _cache:
        _cache["nc"] = _build_nc()
    return _cache["nc"]


def make_mask() -> np.ndarray:
    # mask[j, k, m] = 1 iff partition k of pair-tile j feeds output support m.
    # Pair j < 12 covers supports (2j, 2j+1): k < 64 -> 2j, k >= 64 -> 2j+1.
    # Slot 12 is the leftover single support 24 on partitions 0..63.
    # Slot 13 is all zeros: weights for the PE keep-warm filler matmuls.
    import ml_dtypes

    mask = np.zeros((NMASK, 128, S), dtype=ml_dtypes.bfloat16)
    for j in range(NPAIR):
        mask[j, 0:64, 2 * j] = 1.0
        mask[j, 64:128, 2 * j + 1] = 1.0
    mask[LO, 0:64, S - 1] = 1.0
    return mask


def make_in_maps(x1: np.ndarray, x2: np.ndarray) -> list[dict]:
    import ml_dtypes

    bf16 = ml_dtypes.bfloat16
    x1 = np.asarray(x1, dtype=np.float32).reshape(B, C, HW)
    x2 = np.asarray(x2, dtype=np.float32).reshape(B, S, C, HW)
    mask = make_mask()
    maps = []
    for i in range(NCORES):
        sl = slice(i * BL, (i + 1) * BL)
        # x1 staged bf16, channel-major, duplicated onto both partition
        # halves so it aligns with the (si c) pair layout
        x1c = np.ascontiguousarray(x1[sl].transpose(1, 0, 2)).astype(bf16)
        x1d = np.ascontiguousarray(np.concatenate([x1c, x1c], axis=0))
        x2c = x2[sl].astype(bf16)
        # doubles: [b, dbl, (si c), (pp p)] so each double-pair DMA reads one
        # fully contiguous 7056B row per partition (halves HWDGE descriptors)
        x2d = np.ascontiguousarray(
            x2c[:, : 2 * NPAIR]
            .reshape(BL, NDBL, 2, 2, C, HW)
            .transpose(0, 1, 3, 4, 2, 5)
            .reshape(BL, NDBL, 128, 2 * HW)
        )
        x2l = np.ascontiguousarray(x2c[:, S - 1])
        maps.append({"x1": x1d, "x2": x2d, "x2lo": x2l, "mask": mask})
    return maps


def gather_out(results: list[dict]) -> np.ndarray:
    return np.concatenate([np.asarray(r["out"]) for r in results], axis=0).astype(
        np.float32
    )


def kernel(x1, x2) -> np.ndarray:
    from concourse.bass_utils import run_bass_kernel_spmd

    nc = get_nc()
    in_maps = make_in_maps(x1, x2)
    res = run_bass_kernel_spmd(nc, in_maps, list(range(NCORES)))
    return gather_out(res.results)


# revision 18
# speedup vs baseline: 1.0518x; 1.0518x over previous
"""Euclidean distance block (retrieval kNN) on 8 TRN2 NeuronCores.

dist[b, s, p] = sqrt(sum_c (x1[b, c, p] - x2[b, s, c, p])^2)   p = spatial (h*w)
out[b] = dist[b].reshape(S * h * w)

Sharding: data-parallel over batch B=32 -> 4 batches per core, no comms.

Design (v3; baseline f32/SWDGE was ~145-166us traced, v2 ~120us):

1. HOST-SIDE bf16 STAGING. The baseline streamed x2 as f32 (45 MB/core) and
   cast f32->bf16 on the SWDGE ring; the subtract was already bf16, so
   pre-casting x2/x1 to bf16 on the host gives identical numerics with HALF
   the HBM read traffic (22.6 MB/core) and removes the cast -> every load is
   a plain HWDGE DMA (sync ring, ~0.6us first byte, no ~6us Q7 warmup).
   x1 is also pre-duplicated on host to [128=(2x64c), BL, HW] so the kernel
   needs no SBUF->SBUF partition duplicate. Output is stored bf16 and
   upcast to f32 on host (rel err budget 2e-2, bf16 adds <0.4%).

2. DOUBLE-PAIR PIPELINE. SBUF partitions carry (support_pair, channel) =
   2*64 = 128. Each DMA covers TWO support pairs [128, 2, HW] (902KB), and
   one DVE subtract (bf16 2x mode, x1 broadcast over the pair dim via a
   stride-0 AP) plus one square (4 doubles on ACT, 2 on DVE per batch;
   GpSimd tensor ops measured 5x slow - never use) process both pairs:
   halving the instruction count halves the per-op dependency/sem latency
   that showed up as 15-20% engine idle at pair granularity. PE mask-
   matmuls accumulate per-support sums over C into [25, 441] PSUM tiles
   (4 spatial quarters), ACT sqrt -> bf16 store on the scalar HWDGE ring
   (loads and stores never share a FIFO). x1 lives in one tile per batch
   (a shared tile's slice writes would WAR-serialize against every
   in-flight subtract read).

3. PE KEEP-WARM FILLERS. TRN2's power manager runs the PE at HALF clock
   (371ns per 441-col matmul) unless it has been continuously busy for
   ~3.4us, full clock (188ns) after. Per-pair bursts (~1.5us) with gaps
   never promote. Fillers = matmuls of a zeroed SBUF tile with whatever
   weights are resident (ldweights=False) accumulated into live PSUM:
   adds 0.0, costs no weight reload, keeps the PE promoted.

4. LDWEIGHTS ELISION. The 4 quarter-matmuls of a pair share one mask; only
   quarter 0 self-loads weights (ldweights=False on the rest elides the
   ~101ns InstLdweights each). _verify_ldw_order() walks the final BIR and
   asserts no foreign weight load lands between a loader and its dependents
   (the Tile scheduler could in principle reorder same-engine matmuls).

5. SHORT TAIL. The last batch computes the half-width leftover support 24
   LAST, quarter-sliced: its 4x56KB loads are the final DMAs and each
   quarter's sub->square->matmul(stop)->sqrt->store chain fires as its
   56KB lands, so the post-last-byte critical path is one 441-wide chain.
"""

import numpy as np

B, S, C, H, W = 32, 25, 64, 42, 42
HW = H * W            # 1764
NCORES = 8
BL = B // NCORES      # 4 batches per core
NPAIR = 12            # full support pairs (24 supports); support 24 leftover
NQ = 4                # spatial quarters
QW = HW // NQ         # 441
NMASK = 14            # 12 pair masks + leftover mask (12) + zero filler (13)
LO = 12               # mask index of the leftover support
ZW = 13               # mask index of the all-zero filler weights

NFILL = 4             # keep-warm fillers per double-pair
FILLW = 441           # filler matmul moving columns
ELIDE_LDW = False     # legalization re-pairs an InstLdweights with every
                      # matmult regardless; LDW overlaps MM execution anyway

# square-engine schedule per double-pair i (A=ACT, D=DVE mult)
SQ_ENG = "ADAADA"
SQ_ENG_LAST = "AADAAD"
NDBL = NPAIR // 2     # double-pairs per batch

_cache = {}


def _build_nc():
    import concourse.bacc as bacc
    import concourse.mybir as mybir
    from concourse.tile import TileContext
    from concourse.bass import MemorySpace, broadcast_tensor_aps

    f32 = mybir.dt.float32
    bf16 = mybir.dt.bfloat16
    Square = mybir.ActivationFunctionType.Square
    Sqrt = mybir.ActivationFunctionType.Sqrt
    sub = mybir.AluOpType.subtract
    mul = mybir.AluOpType.mult

    # Square and Sqrt both live in the "sqrt_and_others" act-function set,
    # but the table-load chooser picks the first set containing each one,
    # alternating two ~2.7us table reloads per batch. Strip the two
    # functions from every other set (contents only - set ids are
    # positional) so one resident table serves the whole kernel.
    _orig_tables = bacc.get_activation_tables

    def _pinned_tables(arch):
        t = _orig_tables(arch)
        for name, fns in t.items():
            if name != "sqrt_and_others":
                fns.discard(Square)
                fns.discard(Sqrt)
        return t

    bacc.get_activation_tables = _pinned_tables
    nc = bacc.Bacc()
    x1 = nc.declare_dram_parameter("x1", [128, BL, HW], bf16, isOutput=False)
    x2 = nc.declare_dram_parameter("x2", [BL, NDBL, 128, 2 * HW], bf16, isOutput=False)
    x2lo = nc.declare_dram_parameter("x2lo", [BL, 64, HW], bf16, isOutput=False)
    mk = nc.declare_dram_parameter("mask", [NMASK, 128, S], bf16, isOutput=False)
    out = nc.declare_dram_parameter("out", [BL, S * HW], bf16, isOutput=True)

    # build-time bookkeeping for _verify_ldw_order
    elide_owner = {}      # elided matmult name -> its weight-loader's name
    filler_names = set()

    def mm(pst_q, w, mov, start, stop, loader=None, skip=False):
        inst = nc.tensor.matmul(
            pst_q, w, mov, start=start, stop=stop, skip_group_check=skip
        )
        if loader is not None and ELIDE_LDW:
            inst.ins.ldweights = False
            elide_owner[inst.ins.name] = loader.ins.name
        return inst

    with TileContext(nc) as tc:
        with (
            tc.tile_pool(name="x2p", bufs=8) as x2p,
            tc.tile_pool(name="lop", bufs=2) as lop,
            tc.tile_pool(name="sqp", bufs=4) as sqp,
            tc.tile_pool(name="sqlp", bufs=2) as sqlp,
            tc.tile_pool(name="x1p", bufs=4) as x1p,
            tc.tile_pool(name="outp", bufs=2) as outp,
            tc.tile_pool(name="cst", bufs=1) as cst,
            tc.tile_pool(name="ps", bufs=2, space=MemorySpace.PSUM) as psp,
        ):
            mt = cst.tile([128, NMASK, S], bf16)
            nc.scalar.dma_start(mt[:], mk.rearrange("g k m -> k g m"))

            zt = cst.tile([128, FILLW], bf16, name="zt")
            nc.vector.memset(zt[:], 0.0)

            x1bt = [x1p.tile([128, HW], bf16, name=f"x1b{b}", tag="x1") for b in range(BL)]
            nc.scalar.dma_start(x1bt[0][:], x1[:, 0, :])

            def filler(pst, j, n=NFILL):
                # zero-data accumulates with whatever weights are resident:
                # keeps the PE busy through the per-pair DMA gap so the power
                # manager holds full clock; adds 0.0 to live PSUM
                for k in range(n):
                    inst = nc.tensor.matmul(
                        pst[(j + k) % NQ][:, :],
                        mt[:, ZW, :],
                        zt[:, :],
                        start=False,
                        stop=False,
                        skip_group_check=True,
                    )
                    inst.ins.ldweights = False
                    filler_names.add(inst.ins.name)

            def batch_groups(b):
                # work groups: (first_pair_j, n_pairs, sq_engine). Batch 0
                # opens with two single pairs so the first subtract fires
                # ~2.5us earlier (half the first DMA's wire time).
                sq_eng = SQ_ENG_LAST if b == BL - 1 else SQ_ENG
                if b == 0:
                    return [(0, 1, "A"), (1, 1, "D")] + [
                        (2 * i, 2, sq_eng[i]) for i in range(1, NDBL)
                    ]
                return [(2 * i, 2, sq_eng[i]) for i in range(NDBL)]

            def emit_loads(b):
                # x1 first (needed by the batch's first subtract), doubles
                # alternating across both HWDGE rings, leftover last
                last = b == BL - 1
                if b > 0:
                    nc.scalar.dma_start(x1bt[b][:], x1[:, b, :])
                dbls = []
                for gi, (j0, np_, _) in enumerate(batch_groups(b)):
                    x2t = x2p.tile([128, np_, HW], bf16, tag="x2t")
                    src = x2[b, j0 // 2].rearrange("k (pp p) -> k pp p", pp=2)
                    pp0 = j0 % 2
                    ring = nc.sync if gi % 2 == 0 else nc.scalar
                    ring.dma_start(x2t[:], src[:, pp0 : pp0 + np_, :])
                    dbls.append(x2t)
                x2l = lop.tile([64, HW], bf16, tag="lo")
                if not last:
                    nc.sync.dma_start(x2l[:], x2lo[b])
                else:
                    # leftover is the kernel tail: quarter-sliced, loaded last
                    for q in range(NQ):
                        nc.sync.dma_start(
                            x2l[:, q * QW : (q + 1) * QW],
                            x2lo[b][:, q * QW : (q + 1) * QW],
                        )
                return dbls, x2l

            pending = emit_loads(0)
            for b in range(BL):
                last = b == BL - 1
                groups = batch_groups(b)
                dbls, x2l = pending

                pst = [
                    psp.tile([S, QW], f32, name=f"ps{q}", tag=f"ps{q}")
                    for q in range(NQ)
                ]

                for gi, (j0, np_, eng) in enumerate(groups):
                    x2t = dbls[gi]
                    if np_ == 2:
                        x1u = x1bt[b][:].rearrange("k (u p) -> k u p", u=1)
                        i0, i1 = broadcast_tensor_aps(x2t[:, :, :], x1u)
                    else:
                        i0, i1 = x2t[:, 0, :], x1bt[b][:]
                    nc.vector.tensor_tensor(i0, i0, i1, sub)
                    sq = sqp.tile([128, 2, HW], bf16, tag="sq")
                    if eng == "A":
                        nc.scalar.activation(sq[:, :np_, :], x2t[:], Square)
                    else:
                        nc.vector.tensor_tensor(sq[:, :np_, :], x2t[:], x2t[:], mul)
                    for pi in range(np_):
                        j = j0 + pi
                        for q in range(NQ):
                            mm(
                                pst[q][:, :],
                                mt[:, j, :],
                                sq[:, pi, q * QW : (q + 1) * QW],
                                start=(j == 0),
                                stop=False,
                            )
                    if not last:
                        filler(pst, gi)

                # software-pipelined DMA issue: the next batch's loads are
                # queued on the rings BEFORE this batch's store, so the
                # store's sqrt-wait can never stall them in the ring FIFO
                if not last:
                    pending = emit_loads(b + 1)

                # leftover support 24 last: a short half-width end-chain per
                # batch; its sqrt/store overlaps the next batch's stream
                ot = outp.tile([S, HW], bf16, name="ot", tag="ot")
                sql = sqlp.tile([64, HW], bf16, name="sql", tag="sql")
                if not last:
                    nc.vector.tensor_tensor(x2l[:], x2l[:], x1bt[b][0:64, :], sub)
                    nc.scalar.activation(sql[:], x2l[:], Square)
                    lo_loader = None
                    for q in range(NQ):
                        inst = mm(
                            pst[q][:, :],
                            mt[0:64, LO, :],
                            sql[:, q * QW : (q + 1) * QW],
                            start=False,
                            stop=True,
                            loader=lo_loader,
                        )
                        if lo_loader is None:
                            lo_loader = inst
                    filler(pst, 0, 2)
                    for q in range(NQ):
                        nc.scalar.activation(
                            ot[:, q * QW : (q + 1) * QW], pst[q][:], Sqrt
                        )
                    nc.scalar.dma_start(out[b].rearrange("(s p) -> s p", s=S), ot[:])
                else:
                    # tail: leftover quarters stream in as the final DMAs;
                    # each quarter's chain fires on its own 56KB completion
                    lo_loader = None
                    for q in range(NQ):
                        qs = slice(q * QW, (q + 1) * QW)
                        nc.vector.tensor_tensor(
                            x2l[:, qs], x2l[:, qs], x1bt[b][0:64, qs], sub
                        )
                        nc.scalar.activation(sql[:, qs], x2l[:, qs], Square)
                        inst = mm(
                            pst[q][:, :],
                            mt[0:64, LO, :],
                            sql[:, qs],
                            start=False,
                            stop=True,
                            loader=lo_loader,
                        )
                        if lo_loader is None:
                            lo_loader = inst
                        nc.scalar.activation(ot[:, qs], pst[q][:], Sqrt)
                        nc.scalar.dma_start(
                            out[b].rearrange("(s p) -> s p", s=S)[:, qs], ot[:, qs]
                        )

    try:
        nc.finalize()
    finally:
        bacc.get_activation_tables = _orig_tables
    if ELIDE_LDW:
        _verify_ldw_order(nc, elide_owner, filler_names)
    return nc


def _verify_ldw_order(nc, elide_owner, filler_names):
    """The 4 quarter-matmuls of a pair share one weight load. Walk the final
    (post-Tile-scheduling) program order and assert no other weight-loading
    matmult lands between a loader and its elided dependents."""
    last_loader = None
    for blk in nc.m.functions[0].blocks:
        for inst in blk.instructions:
            if type(inst).__name__ != "InstMatmult":
                continue
            name = inst.name
            if name in filler_names:
                continue  # zero moving data: any weights give 0
            if name in elide_owner:
                if last_loader != elide_owner[name]:
                    raise RuntimeError(
                        f"ldweights elision unsafe: {name} expects weights of "
                        f"{elide_owner[name]} but last loader is {last_loader}"
                    )
            else:
                last_loader = name


def get_nc():
    if "nc" not in _cache:
        _cache["nc"] = _build_nc()
    return _cache["nc"]


def make_mask() -> np.ndarray:
    # mask[j, k, m] = 1 iff partition k of pair-tile j feeds output support m.
    # Pair j < 12 covers supports (2j, 2j+1): k < 64 -> 2j, k >= 64 -> 2j+1.
    # Slot 12 is the leftover single support 24 on partitions 0..63.
    # Slot 13 is all zeros: weights for the PE keep-warm filler matmuls.
    import ml_dtypes

    mask = np.zeros((NMASK, 128, S), dtype=ml_dtypes.bfloat16)
    for j in range(NPAIR):
        mask[j, 0:64, 2 * j] = 1.0
        mask[j, 64:128, 2 * j + 1] = 1.0
    mask[LO, 0:64, S - 1] = 1.0
    return mask


def make_in_maps(x1: np.ndarray, x2: np.ndarray) -> list[dict]:
    import ml_dtypes

    bf16 = ml_dtypes.bfloat16
    x1 = np.asarray(x1, dtype=np.float32).reshape(B, C, HW)
    x2 = np.asarray(x2, dtype=np.float32).reshape(B, S, C, HW)
    mask = make_mask()
    maps = []
    for i in range(NCORES):
        sl = slice(i * BL, (i + 1) * BL)
        # x1 staged bf16, channel-major, duplicated onto both partition
        # halves so it aligns with the (si c) pair layout
        x1c = np.ascontiguousarray(x1[sl].transpose(1, 0, 2)).astype(bf16)
        x1d = np.ascontiguousarray(np.concatenate([x1c, x1c], axis=0))
        x2c = x2[sl].astype(bf16)
        # doubles: [b, dbl, (si c), (pp p)] so each double-pair DMA reads one
        # fully contiguous 7056B row per partition (halves HWDGE descriptors)
        x2d = np.ascontiguousarray(
            x2c[:, : 2 * NPAIR]
            .reshape(BL, NDBL, 2, 2, C, HW)
            .transpose(0, 1, 3, 4, 2, 5)
            .reshape(BL, NDBL, 128, 2 * HW)
        )
        x2l = np.ascontiguousarray(x2c[:, S - 1])
        maps.append({"x1": x1d, "x2": x2d, "x2lo": x2l, "mask": mask})
    return maps


def gather_out(results: list[dict]) -> np.ndarray:
    return np.concatenate([np.asarray(r["out"]) for r in results], axis=0).astype(
        np.float32
    )


def kernel(x1, x2) -> np.ndarray:
    from concourse.bass_utils import run_bass_kernel_spmd

    nc = get_nc()
    in_maps = make_in_maps(x1, x2)
    res = run_bass_kernel_spmd(nc, in_maps, list(range(NCORES)))
    return gather_out(res.results)


# revision 19
# speedup vs baseline: 1.2187x; 1.1587x over previous
"""Euclidean distance block (retrieval kNN) on 8 TRN2 NeuronCores.

dist[b, s, p] = sqrt(sum_c (x1[b, c, p] - x2[b, s, c, p])^2)   p = spatial (h*w)
out[b] = dist[b].reshape(S * h * w)

Sharding: data-parallel over batch B=32 -> 4 batches per core, no comms.

Design (v3; baseline f32/SWDGE was ~145-166us traced, v2 ~120us):

1. HOST-SIDE bf16 STAGING. The baseline streamed x2 as f32 (45 MB/core) and
   cast f32->bf16 on the SWDGE ring; the subtract was already bf16, so
   pre-casting x2/x1 to bf16 on the host gives identical numerics with HALF
   the HBM read traffic (22.6 MB/core) and removes the cast -> every load is
   a plain HWDGE DMA (sync ring, ~0.6us first byte, no ~6us Q7 warmup).
   x1 is also pre-duplicated on host to [128=(2x64c), BL, HW] so the kernel
   needs no SBUF->SBUF partition duplicate. Output is stored bf16 and
   upcast to f32 on host (rel err budget 2e-2, bf16 adds <0.4%).

2. DOUBLE-PAIR PIPELINE. SBUF partitions carry (support_pair, channel) =
   2*64 = 128. Each DMA covers TWO support pairs [128, 2, HW] (902KB), and
   one DVE subtract (bf16 2x mode, x1 broadcast over the pair dim via a
   stride-0 AP) plus one square (4 doubles on ACT, 2 on DVE per batch;
   GpSimd tensor ops measured 5x slow - never use) process both pairs:
   halving the instruction count halves the per-op dependency/sem latency
   that showed up as 15-20% engine idle at pair granularity. PE mask-
   matmuls accumulate per-support sums over C into [25, 441] PSUM tiles
   (4 spatial quarters), ACT sqrt -> bf16 store on the scalar HWDGE ring
   (loads and stores never share a FIFO). x1 lives in one tile per batch
   (a shared tile's slice writes would WAR-serialize against every
   in-flight subtract read).

3. PE KEEP-WARM FILLERS. TRN2's power manager runs the PE at HALF clock
   (371ns per 441-col matmul) unless it has been continuously busy for
   ~3.4us, full clock (188ns) after. Per-pair bursts (~1.5us) with gaps
   never promote. Fillers = matmuls of a zeroed SBUF tile with whatever
   weights are resident (ldweights=False) accumulated into live PSUM:
   adds 0.0, costs no weight reload, keeps the PE promoted.

4. LDWEIGHTS ELISION. The 4 quarter-matmuls of a pair share one mask; only
   quarter 0 self-loads weights (ldweights=False on the rest elides the
   ~101ns InstLdweights each). _verify_ldw_order() walks the final BIR and
   asserts no foreign weight load lands between a loader and its dependents
   (the Tile scheduler could in principle reorder same-engine matmuls).

5. SHORT TAIL. The last batch computes the half-width leftover support 24
   LAST, quarter-sliced: its 4x56KB loads are the final DMAs and each
   quarter's sub->square->matmul(stop)->sqrt->store chain fires as its
   56KB lands, so the post-last-byte critical path is one 441-wide chain.
"""

import numpy as np

B, S, C, H, W = 32, 25, 64, 42, 42
HW = H * W            # 1764
NCORES = 8
BL = B // NCORES      # 4 batches per core
NPAIR = 12            # full support pairs (24 supports); support 24 leftover
NQ = 4                # spatial quarters
QW = HW // NQ         # 441
NMASK = 14            # 12 pair masks + leftover mask (12) + zero filler (13)
LO = 12               # mask index of the leftover support
ZW = 13               # mask index of the all-zero filler weights

NFILL = 4             # keep-warm fillers per double-pair
FILLW = 441           # filler matmul moving columns
ELIDE_LDW = False     # legalization re-pairs an InstLdweights with every
                      # matmult regardless; LDW overlaps MM execution anyway

# square-engine schedule per double-pair i (A=ACT, D=DVE mult)
SQ_ENG = "ADAADA"
SQ_ENG_LAST = "AADAAD"
NDBL = NPAIR // 2     # double-pairs per batch

_cache = {}


def _build_nc():
    import concourse.bacc as bacc
    import concourse.mybir as mybir
    from concourse.tile import TileContext
    from concourse.bass import MemorySpace, broadcast_tensor_aps

    f32 = mybir.dt.float32
    bf16 = mybir.dt.bfloat16
    Square = mybir.ActivationFunctionType.Square
    Sqrt = mybir.ActivationFunctionType.Sqrt
    sub = mybir.AluOpType.subtract
    mul = mybir.AluOpType.mult

    # Square and Sqrt both live in the "sqrt_and_others" act-function set,
    # but the table-load chooser picks the first set containing each one,
    # alternating two ~2.7us table reloads per batch. Strip the two
    # functions from every other set (contents only - set ids are
    # positional) so one resident table serves the whole kernel.
    _orig_tables = bacc.get_activation_tables

    def _pinned_tables(arch):
        t = _orig_tables(arch)
        for name, fns in t.items():
            if name != "sqrt_and_others":
                fns.discard(Square)
                fns.discard(Sqrt)
        return t

    bacc.get_activation_tables = _pinned_tables
    nc = bacc.Bacc()
    x1 = nc.declare_dram_parameter("x1", [128, BL, HW], bf16, isOutput=False)
    x2 = nc.declare_dram_parameter("x2", [BL, NDBL, 128, 2 * HW], bf16, isOutput=False)
    x2lo = nc.declare_dram_parameter("x2lo", [BL, 64, HW], bf16, isOutput=False)
    mk = nc.declare_dram_parameter("mask", [NMASK, 128, S], bf16, isOutput=False)
    out = nc.declare_dram_parameter("out", [BL, S * HW], bf16, isOutput=True)

    # build-time bookkeeping for _verify_ldw_order
    elide_owner = {}      # elided matmult name -> its weight-loader's name
    filler_names = set()

    def mm(pst_q, w, mov, start, stop, loader=None, skip=False):
        inst = nc.tensor.matmul(
            pst_q, w, mov, start=start, stop=stop, skip_group_check=skip
        )
        if loader is not None and ELIDE_LDW:
            inst.ins.ldweights = False
            elide_owner[inst.ins.name] = loader.ins.name
        return inst

    with TileContext(nc) as tc:
        with (
            tc.tile_pool(name="x2p", bufs=8) as x2p,
            tc.tile_pool(name="lop", bufs=2) as lop,
            tc.tile_pool(name="sqp", bufs=4) as sqp,
            tc.tile_pool(name="sqlp", bufs=2) as sqlp,
            tc.tile_pool(name="x1p", bufs=4) as x1p,
            tc.tile_pool(name="outp", bufs=2) as outp,
            tc.tile_pool(name="cst", bufs=1) as cst,
            tc.tile_pool(name="ps", bufs=2, space=MemorySpace.PSUM) as psp,
        ):
            mt = cst.tile([128, NMASK, S], bf16)
            nc.scalar.dma_start(mt[:], mk.rearrange("g k m -> k g m"))

            zt = cst.tile([128, FILLW], bf16, name="zt")
            nc.vector.memset(zt[:], 0.0)

            x1bt = [x1p.tile([128, HW], bf16, name=f"x1b{b}", tag="x1") for b in range(BL)]
            nc.scalar.dma_start(x1bt[0][:], x1[:, 0, :])

            def filler(pst, j, n=NFILL):
                # zero-data accumulates with whatever weights are resident:
                # keeps the PE busy through the per-pair DMA gap so the power
                # manager holds full clock; adds 0.0 to live PSUM
                for k in range(n):
                    inst = nc.tensor.matmul(
                        pst[(j + k) % NQ][:, :],
                        mt[:, ZW, :],
                        zt[:, :],
                        start=False,
                        stop=False,
                        skip_group_check=True,
                    )
                    inst.ins.ldweights = False
                    filler_names.add(inst.ins.name)

            def batch_groups(b):
                # work groups: (first_pair_j, n_pairs, sq_engine). Batch 0
                # opens with two single pairs so the first subtract fires
                # ~2.5us earlier (half the first DMA's wire time).
                sq_eng = SQ_ENG_LAST if b == BL - 1 else SQ_ENG
                if b == 0:
                    return [(0, 1, "A"), (1, 1, "D")] + [
                        (2 * i, 2, sq_eng[i]) for i in range(1, NDBL)
                    ]
                return [(2 * i, 2, sq_eng[i]) for i in range(NDBL)]

            def emit_loads(b):
                # x1 first (needed by the batch's first subtract), doubles
                # alternating across both HWDGE rings, leftover last
                last = b == BL - 1
                if b > 0:
                    nc.scalar.dma_start(x1bt[b][:], x1[:, b, :])
                dbls = []
                for gi, (j0, np_, _) in enumerate(batch_groups(b)):
                    x2t = x2p.tile([128, np_, HW], bf16, tag="x2t")
                    src = x2[b, j0 // 2].rearrange("k (pp p) -> k pp p", pp=2)
                    pp0 = j0 % 2
                    # all x2 loads on the sync ring: the scalar ring's
                    # dispatches share the ACT sequencer, so a load's
                    # buffer-free wait there would stall ACT compute
                    nc.sync.dma_start(x2t[:], src[:, pp0 : pp0 + np_, :])
                    dbls.append(x2t)
                x2l = lop.tile([64, HW], bf16, tag="lo")
                if not last:
                    nc.sync.dma_start(x2l[:], x2lo[b])
                else:
                    # leftover is the kernel tail: quarter-sliced, loaded last
                    for q in range(NQ):
                        nc.sync.dma_start(
                            x2l[:, q * QW : (q + 1) * QW],
                            x2lo[b][:, q * QW : (q + 1) * QW],
                        )
                return dbls, x2l

            pending = emit_loads(0)
            for b in range(BL):
                last = b == BL - 1
                groups = batch_groups(b)
                dbls, x2l = pending

                pst = [
                    psp.tile([S, QW], f32, name=f"ps{q}", tag=f"ps{q}")
                    for q in range(NQ)
                ]

                for gi, (j0, np_, eng) in enumerate(groups):
                    x2t = dbls[gi]
                    if np_ == 2:
                        x1u = x1bt[b][:].rearrange("k (u p) -> k u p", u=1)
                        i0, i1 = broadcast_tensor_aps(x2t[:, :, :], x1u)
                    else:
                        i0, i1 = x2t[:, 0, :], x1bt[b][:]
                    nc.vector.tensor_tensor(i0, i0, i1, sub)
                    sq = sqp.tile([128, 2, HW], bf16, tag="sq")
                    if eng == "A":
                        nc.scalar.activation(sq[:, :np_, :], x2t[:], Square)
                    else:
                        nc.vector.tensor_tensor(sq[:, :np_, :], x2t[:], x2t[:], mul)
                    for pi in range(np_):
                        j = j0 + pi
                        for q in range(NQ):
                            mm(
                                pst[q][:, :],
                                mt[:, j, :],
                                sq[:, pi, q * QW : (q + 1) * QW],
                                start=(j == 0),
                                stop=False,
                            )
                    if not last:
                        filler(pst, gi)

                # software-pipelined DMA issue: the next batch's loads are
                # queued on the rings BEFORE this batch's store, so the
                # store's sqrt-wait can never stall them in the ring FIFO
                if not last:
                    pending = emit_loads(b + 1)

                # leftover support 24 last: a short half-width end-chain per
                # batch; its sqrt/store overlaps the next batch's stream
                ot = outp.tile([S, HW], bf16, name="ot", tag="ot")
                sql = sqlp.tile([64, HW], bf16, name="sql", tag="sql")
                if not last:
                    nc.vector.tensor_tensor(x2l[:], x2l[:], x1bt[b][0:64, :], sub)
                    nc.scalar.activation(sql[:], x2l[:], Square)
                    lo_loader = None
                    for q in range(NQ):
                        inst = mm(
                            pst[q][:, :],
                            mt[0:64, LO, :],
                            sql[:, q * QW : (q + 1) * QW],
                            start=False,
                            stop=True,
                            loader=lo_loader,
                        )
                        if lo_loader is None:
                            lo_loader = inst
                    filler(pst, 0, 2)
                    for q in range(NQ):
                        nc.scalar.activation(
                            ot[:, q * QW : (q + 1) * QW], pst[q][:], Sqrt
                        )
                    nc.scalar.dma_start(out[b].rearrange("(s p) -> s p", s=S), ot[:])
                else:
                    # tail: leftover quarters stream in as the final DMAs;
                    # each quarter's chain fires on its own 56KB completion
                    lo_loader = None
                    for q in range(NQ):
                        qs = slice(q * QW, (q + 1) * QW)
                        nc.vector.tensor_tensor(
                            x2l[:, qs], x2l[:, qs], x1bt[b][0:64, qs], sub
                        )
                        nc.scalar.activation(sql[:, qs], x2l[:, qs], Square)
                        inst = mm(
                            pst[q][:, :],
                            mt[0:64, LO, :],
                            sql[:, qs],
                            start=False,
                            stop=True,
                            loader=lo_loader,
                        )
                        if lo_loader is None:
                            lo_loader = inst
                        nc.scalar.activation(ot[:, qs], pst[q][:], Sqrt)
                        nc.scalar.dma_start(
                            out[b].rearrange("(s p) -> s p", s=S)[:, qs], ot[:, qs]
                        )

    try:
        nc.finalize()
    finally:
        bacc.get_activation_tables = _orig_tables
    if ELIDE_LDW:
        _verify_ldw_order(nc, elide_owner, filler_names)
    return nc


def _verify_ldw_order(nc, elide_owner, filler_names):
    """The 4 quarter-matmuls of a pair share one weight load. Walk the final
    (post-Tile-scheduling) program order and assert no other weight-loading
    matmult lands between a loader and its elided dependents."""
    last_loader = None
    for blk in nc.m.functions[0].blocks:
        for inst in blk.instructions:
            if type(inst).__name__ != "InstMatmult":
                continue
            name = inst.name
            if name in filler_names:
                continue  # zero moving data: any weights give 0
            if name in elide_owner:
                if last_loader != elide_owner[name]:
                    raise RuntimeError(
                        f"ldweights elision unsafe: {name} expects weights of "
                        f"{elide_owner[name]} but last loader is {last_loader}"
                    )
            else:
                last_loader = name


def get_nc():
    if "nc" not in _cache:
        _cache["nc"] = _build_nc()
    return _cache["nc"]


def make_mask() -> np.ndarray:
    # mask[j, k, m] = 1 iff partition k of pair-tile j feeds output support m.
    # Pair j < 12 covers supports (2j, 2j+1): k < 64 -> 2j, k >= 64 -> 2j+1.
    # Slot 12 is the leftover single support 24 on partitions 0..63.
    # Slot 13 is all zeros: weights for the PE keep-warm filler matmuls.
    import ml_dtypes

    mask = np.zeros((NMASK, 128, S), dtype=ml_dtypes.bfloat16)
    for j in range(NPAIR):
        mask[j, 0:64, 2 * j] = 1.0
        mask[j, 64:128, 2 * j + 1] = 1.0
    mask[LO, 0:64, S - 1] = 1.0
    return mask


def make_in_maps(x1: np.ndarray, x2: np.ndarray) -> list[dict]:
    import ml_dtypes

    bf16 = ml_dtypes.bfloat16
    x1 = np.asarray(x1, dtype=np.float32).reshape(B, C, HW)
    x2 = np.asarray(x2, dtype=np.float32).reshape(B, S, C, HW)
    mask = make_mask()
    maps = []
    for i in range(NCORES):
        sl = slice(i * BL, (i + 1) * BL)
        # x1 staged bf16, channel-major, duplicated onto both partition
        # halves so it aligns with the (si c) pair layout
        x1c = np.ascontiguousarray(x1[sl].transpose(1, 0, 2)).astype(bf16)
        x1d = np.ascontiguousarray(np.concatenate([x1c, x1c], axis=0))
        x2c = x2[sl].astype(bf16)
        # doubles: [b, dbl, (si c), (pp p)] so each double-pair DMA reads one
        # fully contiguous 7056B row per partition (halves HWDGE descriptors)
        x2d = np.ascontiguousarray(
            x2c[:, : 2 * NPAIR]
            .reshape(BL, NDBL, 2, 2, C, HW)
            .transpose(0, 1, 3, 4, 2, 5)
            .reshape(BL, NDBL, 128, 2 * HW)
        )
        x2l = np.ascontiguousarray(x2c[:, S - 1])
        maps.append({"x1": x1d, "x2": x2d, "x2lo": x2l, "mask": mask})
    return maps


def gather_out(results: list[dict]) -> np.ndarray:
    return np.concatenate([np.asarray(r["out"]) for r in results], axis=0).astype(
        np.float32
    )


def kernel(x1, x2) -> np.ndarray:
    from concourse.bass_utils import run_bass_kernel_spmd

    nc = get_nc()
    in_maps = make_in_maps(x1, x2)
    res = run_bass_kernel_spmd(nc, in_maps, list(range(NCORES)))
    return gather_out(res.results)


# revision 20
# speedup vs baseline: 1.2542x; 1.0292x over previous
"""Euclidean distance block (retrieval kNN) on 8 TRN2 NeuronCores.

dist[b, s, p] = sqrt(sum_c (x1[b, c, p] - x2[b, s, c, p])^2)   p = spatial (h*w)
out[b] = dist[b].reshape(S * h * w)

Sharding: data-parallel over batch B=32 -> 4 batches per core, no comms.

Design (v3; baseline f32/SWDGE was ~145-166us traced, v2 ~120us):

1. HOST-SIDE bf16 STAGING. The baseline streamed x2 as f32 (45 MB/core) and
   cast f32->bf16 on the SWDGE ring; the subtract was already bf16, so
   pre-casting x2/x1 to bf16 on the host gives identical numerics with HALF
   the HBM read traffic (22.6 MB/core) and removes the cast -> every load is
   a plain HWDGE DMA (sync ring, ~0.6us first byte, no ~6us Q7 warmup).
   x1 is also pre-duplicated on host to [128=(2x64c), BL, HW] so the kernel
   needs no SBUF->SBUF partition duplicate. Output is stored bf16 and
   upcast to f32 on host (rel err budget 2e-2, bf16 adds <0.4%).

2. DOUBLE-PAIR PIPELINE. SBUF partitions carry (support_pair, channel) =
   2*64 = 128. Each DMA covers TWO support pairs [128, 2, HW] (902KB), and
   one DVE subtract (bf16 2x mode, x1 broadcast over the pair dim via a
   stride-0 AP) plus one square (4 doubles on ACT, 2 on DVE per batch;
   GpSimd tensor ops measured 5x slow - never use) process both pairs:
   halving the instruction count halves the per-op dependency/sem latency
   that showed up as 15-20% engine idle at pair granularity. PE mask-
   matmuls accumulate per-support sums over C into [25, 441] PSUM tiles
   (4 spatial quarters), ACT sqrt -> bf16 store on the scalar HWDGE ring
   (loads and stores never share a FIFO). x1 lives in one tile per batch
   (a shared tile's slice writes would WAR-serialize against every
   in-flight subtract read).

3. PE KEEP-WARM FILLERS. TRN2's power manager runs the PE at HALF clock
   (371ns per 441-col matmul) unless it has been continuously busy for
   ~3.4us, full clock (188ns) after. Per-pair bursts (~1.5us) with gaps
   never promote. Fillers = matmuls of a zeroed SBUF tile with whatever
   weights are resident (ldweights=False) accumulated into live PSUM:
   adds 0.0, costs no weight reload, keeps the PE promoted.

4. LDWEIGHTS ELISION. The 4 quarter-matmuls of a pair share one mask; only
   quarter 0 self-loads weights (ldweights=False on the rest elides the
   ~101ns InstLdweights each). _verify_ldw_order() walks the final BIR and
   asserts no foreign weight load lands between a loader and its dependents
   (the Tile scheduler could in principle reorder same-engine matmuls).

5. SHORT TAIL. The last batch computes the half-width leftover support 24
   LAST, quarter-sliced: its 4x56KB loads are the final DMAs and each
   quarter's sub->square->matmul(stop)->sqrt->store chain fires as its
   56KB lands, so the post-last-byte critical path is one 441-wide chain.
"""

import numpy as np

B, S, C, H, W = 32, 25, 64, 42, 42
HW = H * W            # 1764
NCORES = 8
BL = B // NCORES      # 4 batches per core
NPAIR = 12            # full support pairs (24 supports); support 24 leftover
NQ = 4                # spatial quarters
QW = HW // NQ         # 441
NMASK = 14            # 12 pair masks + leftover mask (12) + zero filler (13)
LO = 12               # mask index of the leftover support
ZW = 13               # mask index of the all-zero filler weights

NFILL = 2             # keep-warm fillers per double-pair
FILLW = 441           # filler matmul moving columns
ELIDE_LDW = False     # legalization re-pairs an InstLdweights with every
                      # matmult regardless; LDW overlaps MM execution anyway

# square-engine schedule per double-pair i (A=ACT, D=DVE mult)
SQ_ENG = "ADAADA"
SQ_ENG_LAST = "AADAAD"
NDBL = NPAIR // 2     # double-pairs per batch

_cache = {}


def _build_nc():
    import concourse.bacc as bacc
    import concourse.mybir as mybir
    from concourse.tile import TileContext
    from concourse.bass import MemorySpace, broadcast_tensor_aps

    f32 = mybir.dt.float32
    bf16 = mybir.dt.bfloat16
    Square = mybir.ActivationFunctionType.Square
    Sqrt = mybir.ActivationFunctionType.Sqrt
    sub = mybir.AluOpType.subtract
    mul = mybir.AluOpType.mult

    # Square and Sqrt both live in the "sqrt_and_others" act-function set,
    # but the table-load chooser picks the first set containing each one,
    # alternating two ~2.7us table reloads per batch. Strip the two
    # functions from every other set (contents only - set ids are
    # positional) so one resident table serves the whole kernel.
    _orig_tables = bacc.get_activation_tables

    def _pinned_tables(arch):
        t = _orig_tables(arch)
        for name, fns in t.items():
            if name != "sqrt_and_others":
                fns.discard(Square)
                fns.discard(Sqrt)
        return t

    bacc.get_activation_tables = _pinned_tables
    nc = bacc.Bacc()
    x1 = nc.declare_dram_parameter("x1", [128, BL, HW], bf16, isOutput=False)
    x2 = nc.declare_dram_parameter("x2", [BL, NDBL, 128, 2 * HW], bf16, isOutput=False)
    x2lo = nc.declare_dram_parameter("x2lo", [BL, 64, HW], bf16, isOutput=False)
    mk = nc.declare_dram_parameter("mask", [NMASK, 128, S], bf16, isOutput=False)
    out = nc.declare_dram_parameter("out", [BL, S * HW], bf16, isOutput=True)

    # build-time bookkeeping for _verify_ldw_order
    elide_owner = {}      # elided matmult name -> its weight-loader's name
    filler_names = set()

    def mm(pst_q, w, mov, start, stop, loader=None, skip=False):
        inst = nc.tensor.matmul(
            pst_q, w, mov, start=start, stop=stop, skip_group_check=skip
        )
        if loader is not None and ELIDE_LDW:
            inst.ins.ldweights = False
            elide_owner[inst.ins.name] = loader.ins.name
        return inst

    with TileContext(nc) as tc:
        with (
            tc.tile_pool(name="x2p", bufs=8) as x2p,
            tc.tile_pool(name="lop", bufs=2) as lop,
            tc.tile_pool(name="sqp", bufs=4) as sqp,
            tc.tile_pool(name="sqlp", bufs=2) as sqlp,
            tc.tile_pool(name="x1p", bufs=4) as x1p,
            tc.tile_pool(name="outp", bufs=2) as outp,
            tc.tile_pool(name="cst", bufs=1) as cst,
            tc.tile_pool(name="ps", bufs=2, space=MemorySpace.PSUM) as psp,
        ):
            mt = cst.tile([128, NMASK, S], bf16)
            nc.scalar.dma_start(mt[:], mk.rearrange("g k m -> k g m"))

            zt = cst.tile([128, FILLW], bf16, name="zt")
            nc.vector.memset(zt[:], 0.0)

            x1bt = [x1p.tile([128, HW], bf16, name=f"x1b{b}", tag="x1") for b in range(BL)]
            nc.scalar.dma_start(x1bt[0][:], x1[:, 0, :])

            def filler(pst, j, n=NFILL):
                # zero-data accumulates with whatever weights are resident:
                # keeps the PE busy through the per-pair DMA gap so the power
                # manager holds full clock; adds 0.0 to live PSUM
                for k in range(n):
                    inst = nc.tensor.matmul(
                        pst[(j + k) % NQ][:, :],
                        mt[:, ZW, :],
                        zt[:, :],
                        start=False,
                        stop=False,
                        skip_group_check=True,
                    )
                    inst.ins.ldweights = False
                    filler_names.add(inst.ins.name)

            def batch_groups(b):
                # work groups: (first_pair_j, n_pairs, sq_engine). Batch 0
                # opens with two single pairs so the first subtract fires
                # ~2.5us earlier (half the first DMA's wire time).
                sq_eng = SQ_ENG_LAST if b == BL - 1 else SQ_ENG
                if b == 0:
                    return [(0, 1, "A"), (1, 1, "D")] + [
                        (2 * i, 2, sq_eng[i]) for i in range(1, NDBL)
                    ]
                return [(2 * i, 2, sq_eng[i]) for i in range(NDBL)]

            def emit_loads(b):
                # x1 first (needed by the batch's first subtract), doubles
                # alternating across both HWDGE rings, leftover last
                last = b == BL - 1
                if b > 0:
                    nc.scalar.dma_start(x1bt[b][:], x1[:, b, :])
                dbls = []
                for gi, (j0, np_, _) in enumerate(batch_groups(b)):
                    x2t = x2p.tile([128, np_, HW], bf16, tag="x2t")
                    src = x2[b, j0 // 2].rearrange("k (pp p) -> k pp p", pp=2)
                    pp0 = j0 % 2
                    # all x2 loads on the sync ring: the scalar ring's
                    # dispatches share the ACT sequencer, so a load's
                    # buffer-free wait there would stall ACT compute
                    nc.sync.dma_start(x2t[:], src[:, pp0 : pp0 + np_, :])
                    dbls.append(x2t)
                x2l = lop.tile([64, HW], bf16, tag="lo")
                if not last:
                    nc.sync.dma_start(x2l[:], x2lo[b])
                else:
                    # leftover is the kernel tail: quarter-sliced, loaded last
                    for q in range(NQ):
                        nc.sync.dma_start(
                            x2l[:, q * QW : (q + 1) * QW],
                            x2lo[b][:, q * QW : (q + 1) * QW],
                        )
                return dbls, x2l

            pending = emit_loads(0)
            for b in range(BL):
                last = b == BL - 1
                groups = batch_groups(b)
                dbls, x2l = pending

                pst = [
                    psp.tile([S, QW], f32, name=f"ps{q}", tag=f"ps{q}")
                    for q in range(NQ)
                ]

                for gi, (j0, np_, eng) in enumerate(groups):
                    x2t = dbls[gi]
                    if np_ == 2:
                        x1u = x1bt[b][:].rearrange("k (u p) -> k u p", u=1)
                        i0, i1 = broadcast_tensor_aps(x2t[:, :, :], x1u)
                    else:
                        i0, i1 = x2t[:, 0, :], x1bt[b][:]
                    nc.vector.tensor_tensor(i0, i0, i1, sub)
                    sq = sqp.tile([128, 2, HW], bf16, tag="sq")
                    if eng == "A":
                        nc.scalar.activation(sq[:, :np_, :], x2t[:], Square)
                    else:
                        nc.vector.tensor_tensor(sq[:, :np_, :], x2t[:], x2t[:], mul)
                    for pi in range(np_):
                        j = j0 + pi
                        for q in range(NQ):
                            mm(
                                pst[q][:, :],
                                mt[:, j, :],
                                sq[:, pi, q * QW : (q + 1) * QW],
                                start=(j == 0),
                                stop=False,
                            )
                    if not last:
                        filler(pst, gi)

                # software-pipelined DMA issue: the next batch's loads are
                # queued on the rings BEFORE this batch's store, so the
                # store's sqrt-wait can never stall them in the ring FIFO
                if not last:
                    pending = emit_loads(b + 1)

                # leftover support 24 last: a short half-width end-chain per
                # batch; its sqrt/store overlaps the next batch's stream
                ot = outp.tile([S, HW], bf16, name="ot", tag="ot")
                sql = sqlp.tile([64, HW], bf16, name="sql", tag="sql")
                if not last:
                    nc.vector.tensor_tensor(x2l[:], x2l[:], x1bt[b][0:64, :], sub)
                    nc.scalar.activation(sql[:], x2l[:], Square)
                    lo_loader = None
                    for q in range(NQ):
                        inst = mm(
                            pst[q][:, :],
                            mt[0:64, LO, :],
                            sql[:, q * QW : (q + 1) * QW],
                            start=False,
                            stop=True,
                            loader=lo_loader,
                        )
                        if lo_loader is None:
                            lo_loader = inst
                    filler(pst, 0, 2)
                    for q in range(NQ):
                        nc.scalar.activation(
                            ot[:, q * QW : (q + 1) * QW], pst[q][:], Sqrt
                        )
                    nc.scalar.dma_start(out[b].rearrange("(s p) -> s p", s=S), ot[:])
                else:
                    # tail: leftover quarters stream in as the final DMAs;
                    # each quarter's chain fires on its own 56KB completion
                    lo_loader = None
                    for q in range(NQ):
                        qs = slice(q * QW, (q + 1) * QW)
                        nc.vector.tensor_tensor(
                            x2l[:, qs], x2l[:, qs], x1bt[b][0:64, qs], sub
                        )
                        nc.scalar.activation(sql[:, qs], x2l[:, qs], Square)
                        inst = mm(
                            pst[q][:, :],
                            mt[0:64, LO, :],
                            sql[:, qs],
                            start=False,
                            stop=True,
                            loader=lo_loader,
                        )
                        if lo_loader is None:
                            lo_loader = inst
                        nc.scalar.activation(ot[:, qs], pst[q][:], Sqrt)
                        nc.scalar.dma_start(
                            out[b].rearrange("(s p) -> s p", s=S)[:, qs], ot[:, qs]
                        )

    try:
        nc.finalize()
    finally:
        bacc.get_activation_tables = _orig_tables
    if ELIDE_LDW:
        _verify_ldw_order(nc, elide_owner, filler_names)
    return nc


def _verify_ldw_order(nc, elide_owner, filler_names):
    """The 4 quarter-matmuls of a pair share one weight load. Walk the final
    (post-Tile-scheduling) program order and assert no other weight-loading
    matmult lands between a loader and its elided dependents."""
    last_loader = None
    for blk in nc.m.functions[0].blocks:
        for inst in blk.instructions:
            if type(inst).__name__ != "InstMatmult":
                continue
            name = inst.name
            if name in filler_names:
                continue  # zero moving data: any weights give 0
            if name in elide_owner:
                if last_loader != elide_owner[name]:
                    raise RuntimeError(
                        f"ldweights elision unsafe: {name} expects weights of "
                        f"{elide_owner[name]} but last loader is {last_loader}"
                    )
            else:
                last_loader = name


def get_nc():
    if "nc" not in _cache:
        _cache["nc"] = _build_nc()
    return _cache["nc"]


def make_mask() -> np.ndarray:
    # mask[j, k, m] = 1 iff partition k of pair-tile j feeds output support m.
    # Pair j < 12 covers supports (2j, 2j+1): k < 64 -> 2j, k >= 64 -> 2j+1.
    # Slot 12 is the leftover single support 24 on partitions 0..63.
    # Slot 13 is all zeros: weights for the PE keep-warm filler matmuls.
    import ml_dtypes

    mask = np.zeros((NMASK, 128, S), dtype=ml_dtypes.bfloat16)
    for j in range(NPAIR):
        mask[j, 0:64, 2 * j] = 1.0
        mask[j, 64:128, 2 * j + 1] = 1.0
    mask[LO, 0:64, S - 1] = 1.0
    return mask


def make_in_maps(x1: np.ndarray, x2: np.ndarray) -> list[dict]:
    import ml_dtypes

    bf16 = ml_dtypes.bfloat16
    x1 = np.asarray(x1, dtype=np.float32).reshape(B, C, HW)
    x2 = np.asarray(x2, dtype=np.float32).reshape(B, S, C, HW)
    mask = make_mask()
    maps = []
    for i in range(NCORES):
        sl = slice(i * BL, (i + 1) * BL)
        # x1 staged bf16, channel-major, duplicated onto both partition
        # halves so it aligns with the (si c) pair layout
        x1c = np.ascontiguousarray(x1[sl].transpose(1, 0, 2)).astype(bf16)
        x1d = np.ascontiguousarray(np.concatenate([x1c, x1c], axis=0))
        x2c = x2[sl].astype(bf16)
        # doubles: [b, dbl, (si c), (pp p)] so each double-pair DMA reads one
        # fully contiguous 7056B row per partition (halves HWDGE descriptors)
        x2d = np.ascontiguousarray(
            x2c[:, : 2 * NPAIR]
            .reshape(BL, NDBL, 2, 2, C, HW)
            .transpose(0, 1, 3, 4, 2, 5)
            .reshape(BL, NDBL, 128, 2 * HW)
        )
        x2l = np.ascontiguousarray(x2c[:, S - 1])
        maps.append({"x1": x1d, "x2": x2d, "x2lo": x2l, "mask": mask})
    return maps


def gather_out(results: list[dict]) -> np.ndarray:
    return np.concatenate([np.asarray(r["out"]) for r in results], axis=0).astype(
        np.float32
    )


def kernel(x1, x2) -> np.ndarray:
    from concourse.bass_utils import run_bass_kernel_spmd

    nc = get_nc()
    in_maps = make_in_maps(x1, x2)
    res = run_bass_kernel_spmd(nc, in_maps, list(range(NCORES)))
    return gather_out(res.results)
